# revision 1
# baseline (speedup 1.0000x reference)
"""APPNP GNN kernel for 8 Trainium2 NeuronCores.

Pipeline: h = gelu(x@W1+b1); h = LN(h,g1,be1); h = APPNP_K10(h); h = gelu(h);
h = LN(h,g2,be2); out = h@W2+b2.

Strategy: destination-sharded nodes (12500/core, padded 12544 = 98 tiles of
128). Per hop, with g = dinv*h folded so the segment-sum is unweighted:
    h'[d] = (1-a)*dinv[d]*(sum_{e->d} g[src_e] + g[d]) + a*h0[d]
g is kept in bf16. Each hop AllGathers the local g shard into a replicated
g_full HBM buffer (partition-contiguous layout), then dma_gather pulls
256-byte PAIRS of g rows per edge (int16 indices address 512B superrows of 4
nodes; 2 residue windows select the superrow half). The segment-sum runs on
the TensorEngine: per 128-edge chunk two banded 0/1 matmuls (even/odd source
parity) accumulate into the PSUM 32-strip of the destination tile. The chunk
grid (tile x window x 32-strip quotas) is computed on the host from ALL
cores' edge counts so the traced program is identical across cores (SPMD).
"""
import math
import numpy as np
import ml_dtypes

import concourse.bass as bass
import concourse.bacc as bacc
import concourse.mybir as mybir
import concourse.tile as tile_mod
from concourse.tile import TileContext
from concourse.masks import make_identity

# ---- workaround: walrus CTRL instructions support few sync-wait slots; the
# TileContext exit drain carries one wait per live semaphore. Split excess
# waits onto individual SP nops emitted before a second drain. ----
_MAX_CTRL_WAITS = 1


def _patched_drain_and_barrier(self, tick_clock, wait_clock):
    drain_inst = self.nc.sync.drain()
    wait_clock.add_sem_waits(
        drain_inst.ins, tile_mod.ScopedClock({None: tick_clock.global_clock}))
    si = drain_inst.ins.sync_info
    if si is not None and si.on_wait and len(si.on_wait) > _MAX_CTRL_WAITS:
        waits = list(si.on_wait)
        si.on_wait = waits[:_MAX_CTRL_WAITS]
        for w in waits[_MAX_CTRL_WAITS:]:
            nop = self.nc.sync.nop(nofuse=True, hint="drain_wait_split")
            nsi = nop.ins.sync_info
            if nsi is None:
                nop.ins.sync_info = type(si)(on_wait=[w], on_update=[])
            else:
                nsi.on_wait = [w]
        self.nc.sync.drain()
    self.nc.all_engine_barrier()
    assert self.sems is not None
    popped = self.nc._tile_sem_poison_stack.pop()
    assert popped is self._sem_poison
    self.nc.clear_and_free_semaphores(list(self.sems.allocated().values()))
    self.nc.all_engine_barrier()


TileContext._drain_and_barrier = _patched_drain_and_barrier

FP = mybir.dt.float32
BF = mybir.dt.bfloat16

N, E, IN_C, HID, OUT_C = 100000, 1600000, 256, 64, 64
K_HOPS, ALPHA, LN_EPS = 10, 0.1, 1e-5
N_CORES = 8

TILES_PER_BANK = 7
BANKS = 7          # PSUM banks used per segsum pass (1 left for front/back)
BLOCK_CHUNKS = 8   # chunks per gather call (SWDGE ring caps at 1024 desc/dir)
GATHER_QUEUES = 1  # Tile sem assignment only supports one SWDGE queue
AG_SPLIT = False   # one AllGather per hop (ncfw per-collective fixed cost beats split overlap on HW)


def pack_idx16(idx: np.ndarray) -> np.ndarray:
    """[n] -> [128, n/16] int16 (16-partition wrap, replicated 8x for Q7s)."""
    n = idx.shape[0]
    assert n % 16 == 0
    t = idx.astype(np.int16).reshape(n // 16, 16).T
    return np.tile(t, (8, 1))


class Plan:
    def __init__(self, n_total, e_total):
        self.n_total = n_total
        self.per_core = n_total // N_CORES
        self.sh = ((self.per_core + 127) // 128) * 128
        self.tiles = self.sh // 128
        self.pass_tiles = BANKS * TILES_PER_BANK
        self.n_passes = math.ceil(self.tiles / self.pass_tiles)
        self.rows_full = self.sh * N_CORES
        assert self.rows_full % 4 == 0
        assert (self.rows_full // 4) - 1 <= 32767, "int16 idx overflow"


def preprocess(x, edge_index, plan):
    pc, sh, tiles = plan.per_core, plan.sh, plan.tiles
    src = edge_index[0].astype(np.int64)
    dst = edge_index[1].astype(np.int64)

    deg = np.bincount(dst, minlength=plan.n_total).astype(np.float64) + 1.0
    dinv = (1.0 / np.sqrt(deg)).astype(np.float32)

    c_dst = dst // pc
    ld = dst - c_dst * pc
    e_t, e_p = ld // 128, ld % 128
    e_s = e_p // 32
    c_src = src // pc
    rs = src - c_src * pc
    s_t, s_p = rs // 128, rs % 128
    s_pz = s_t // plan.pass_tiles
    s_tl = s_t - s_pz * plan.pass_tiles
    tp = np.minimum(plan.pass_tiles, tiles - s_pz * plan.pass_tiles)
    gpos = (c_src * sh + s_pz * plan.pass_tiles * 128
            + s_p * tp + s_tl)
    e_w = ((gpos % 4) // 2).astype(np.int64)
    e_par = (gpos % 2).astype(np.int64)
    e_idx = (gpos // 4).astype(np.int16)

    # shared chunk quotas per (tile, window, strip)
    cnt = np.zeros((N_CORES, tiles, 2, 4), np.int64)
    np.add.at(cnt.reshape(-1), ((c_dst * tiles + e_t) * 2 + e_w) * 4 + e_s, 1)
    quota = np.maximum(1, np.ceil(cnt.max(axis=0) / 128.0).astype(np.int64))

    grid = {}  # (pass, bank, w) -> ordered [(tile, strip)] chunk list
    for pz in range(plan.n_passes):
        t0 = pz * plan.pass_tiles
        t1 = min(t0 + plan.pass_tiles, tiles)
        nbank = math.ceil((t1 - t0) / TILES_PER_BANK)
        for b in range(nbank):
            bt0 = t0 + b * TILES_PER_BANK
            bt1 = min(bt0 + TILES_PER_BANK, t1)
            for w in range(2):
                chunks = []
                for t in range(bt0, bt1):
                    for s in range(4):
                        chunks.extend([(t, s)] * int(quota[t, w, s]))
                grid[(pz, b, w)] = chunks

    order = np.lexsort((e_p, e_s, e_w, e_t, c_dst))
    so_idx = e_idx[order]
    so_par = e_par[order].astype(np.int64)
    so_p = e_p[order].astype(np.int64)
    gkey = (((c_dst[order] * tiles + e_t[order]) * 2 + e_w[order]) * 4 + e_s[order])
    n_keys = N_CORES * tiles * 2 * 4
    starts = np.searchsorted(gkey, np.arange(n_keys))
    ends = np.searchsorted(gkey, np.arange(n_keys), side="right")

    # block structure (shared): gather blocks within each (pass,bank,w) group;
    # one S-load per group
    gkeys = sorted(grid.keys())
    blocks = []   # (gkey, chunk_lo, n_chunks, idx_col_off)
    groups = {}   # gkey -> (s_off, nch)
    idx_cols_total, s_size = 0, 0
    for gk in gkeys:
        nch = len(grid[gk])
        groups[gk] = (s_size, nch)
        s_size += 128 * nch * 64
        for lo in range(0, nch, BLOCK_CHUNKS):
            bc = min(BLOCK_CHUNKS, nch - lo)
            blocks.append((gk, lo, bc, idx_cols_total))
            idx_cols_total += bc * 8

    idx_bufs, s_bufs = [], []
    for c in range(N_CORES):
        idx_buf = np.zeros((128, idx_cols_total), np.int16)
        s_flat = np.zeros(s_size, ml_dtypes.bfloat16)
        for gk in gkeys:
            pz, b, w = gk
            chunks = grid[gk]
            nch = len(chunks)
            soff, _ = groups[gk]
            slots = np.zeros((nch, 128), np.int16)
            sm = np.zeros((nch, 128, 64), ml_dtypes.bfloat16)
            ci = 0
            while ci < nch:
                t, s = chunks[ci]
                reps = 1
                while ci + reps < nch and chunks[ci + reps] == (t, s):
                    reps += 1
                key = ((c * tiles + t) * 2 + w) * 4 + s
                s0, s1 = int(starts[key]), int(ends[key])
                npz = s1 - s0
                assert npz <= reps * 128
                if npz:
                    ce = ci + np.arange(npz) // 128        # chunk id
                    sl = np.arange(npz) % 128              # slot in chunk
                    slots[ce, sl] = so_idx[s0:s1]
                    col = (so_p[s0:s1] % 32) + 32 * so_par[s0:s1]
                    sm[ce, sl, col] = 1.0
                ci += reps
            s_flat[soff:soff + 128 * nch * 64] = (
                sm.transpose(1, 0, 2).reshape(-1))
            for (bgk, lo, bc, coff) in blocks:
                if bgk != gk:
                    continue
                idx_buf[:, coff:coff + bc * 8] = pack_idx16(
                    slots[lo:lo + bc].reshape(-1))
        idx_bufs.append(idx_buf)
        s_bufs.append(s_flat)

    dinv_t, dsc_t, x_t = [], [], []
    for c in range(N_CORES):
        dv = np.ones(sh, np.float32)
        dv[:pc] = dinv[c * pc:(c + 1) * pc]
        dinv_t.append(dv.reshape(tiles, 128).T.copy())
        dsc_t.append(((1.0 - ALPHA) * dv).reshape(tiles, 128).T.copy())
        xs = np.zeros((sh, IN_C), np.float32)
        xs[:pc] = x[c * pc:(c + 1) * pc]
        x_t.append(xs.T.copy())
    return {
        "grid": grid, "blocks": blocks, "groups": groups,
        "idx_cols_total": idx_cols_total,
        "s_size": s_size, "idx_bufs": idx_bufs, "s_bufs": s_bufs,
        "dinv_t": dinv_t, "dsc_t": dsc_t, "x_t": x_t,
    }


def build_nc(plan, pre, n_hops=K_HOPS):
    nc = bacc.Bacc("TRN2", num_devices=N_CORES, num_swdge_queues=GATHER_QUEUES)
    sh, tiles = plan.sh, plan.tiles
    d = HID

    xT = nc.dram_tensor("xT", [IN_C, sh], FP, kind="ExternalInput")
    W1 = nc.dram_tensor("W1", [IN_C, HID], FP, kind="ExternalInput")
    b1 = nc.dram_tensor("b1", [HID], FP, kind="ExternalInput")
    g1 = nc.dram_tensor("g1", [HID], FP, kind="ExternalInput")
    be1 = nc.dram_tensor("be1", [HID], FP, kind="ExternalInput")
    W2p = nc.dram_tensor("W2p", [HID, OUT_C], FP, kind="ExternalInput")
    b2p = nc.dram_tensor("b2p", [OUT_C], FP, kind="ExternalInput")
    dinv_d = nc.dram_tensor("dinv", [128, tiles], FP, kind="ExternalInput")
    dsc_d = nc.dram_tensor("dsc", [128, tiles], FP, kind="ExternalInput")
    idx_d = nc.dram_tensor("idxs", [128, pre["idx_cols_total"]], mybir.dt.int16,
                           kind="ExternalInput")
    s_d = nc.dram_tensor("smat", [pre["s_size"]], BF, kind="ExternalInput")
    y = nc.dram_tensor("y", [sh, OUT_C], FP, kind="ExternalOutput")

    ag_in = [nc.dram_tensor(f"ag_in{i}", [sh * d], BF) for i in range(2)]
    g_full = [nc.dram_tensor(f"g_full{i}", [plan.rows_full * d], BF,
                             addr_space="Shared") for i in range(2)]
    rg = [list(range(N_CORES))]

    def bcast_row(pool, dram, width):
        tile = pool.tile([128, width], FP, tag=f"bc_{dram.name}",
                         name=f"bc_{dram.name}")
        ap = bass.AP(tensor=dram, offset=0, ap=[[0, 128], [1, width]])
        nc.gpsimd.dma_start(out=tile[:], in_=ap)
        return tile

    with TileContext(nc) as tc:
        import contextlib
        with contextlib.ExitStack() as ctx:
            const = ctx.enter_context(tc.tile_pool(name="const", bufs=1))
            mpool = ctx.enter_context(tc.tile_pool(name="msg", bufs=8))
            spool = ctx.enter_context(tc.tile_pool(name="smat", bufs=3))
            xpool = ctx.enter_context(tc.tile_pool(name="xt", bufs=3))
            tpool = ctx.enter_context(tc.tile_pool(name="tmp", bufs=6))
            ps_f = ctx.enter_context(tc.tile_pool(name="psf", bufs=1, space="PSUM"))
            ps_s = ctx.enter_context(tc.tile_pool(name="pss", bufs=BANKS, space="PSUM"))

            ident = const.tile([128, 128], FP, tag="ident")
            make_identity(nc, ident[:])
            ident_bf = const.tile([128, 128], BF, tag="identbf")
            nc.vector.tensor_copy(out=ident_bf[:], in_=ident[:])
            eps_t = const.tile([128, 1], FP, tag="eps")
            nc.vector.memset(eps_t[:], LN_EPS)
            b1r = bcast_row(const, b1, HID)
            g1r = bcast_row(const, g1, HID)
            be1r = bcast_row(const, be1, HID)
            b2r = bcast_row(const, b2p, OUT_C)
            W1t = const.tile([128, 2, HID], FP, tag="w1")
            nc.sync.dma_start(out=W1t[:], in_=W1[:].rearrange("(k p) d -> p k d", p=128))
            W2t = const.tile([64, OUT_C], FP, tag="w2")
            nc.sync.dma_start(out=W2t[:], in_=W2p[:])
            dinv_t = const.tile([128, tiles], FP, tag="dinv")
            nc.sync.dma_start(out=dinv_t[:], in_=dinv_d[:])
            dsc_t = const.tile([128, tiles], FP, tag="dsc")
            nc.sync.dma_start(out=dsc_t[:], in_=dsc_d[:])
            idx_t = const.tile([128, pre["idx_cols_total"]], mybir.dt.int16, tag="idx")
            nc.sync.dma_start(out=idx_t[:], in_=idx_d[:])

            h_sb = const.tile([128, tiles, d], FP, tag="h")
            ah0_sb = const.tile([128, tiles, d], FP, tag="ah0")
            g_sb = const.tile([128, tiles, d], BF, tag="g")

            def layernorm(dst_ap, src_ap, gamma_row, beta_row):
                stats = tpool.tile([128, 6], FP, tag="stats", name="stats")
                mv = tpool.tile([128, 2], FP, tag="mv", name="mv")
                nc.vector.bn_stats(out=stats[:], in_=src_ap)
                nc.vector.bn_aggr(out=mv[:], in_=stats[:])
                sd = tpool.tile([128, 1], FP, tag="sd", name="sd")
                nc.scalar.activation(out=sd[:], in_=mv[:, 1:2],
                                     func=mybir.ActivationFunctionType.Sqrt,
                                     bias=eps_t[:], scale=1.0)
                rs = tpool.tile([128, 1], FP, tag="rs", name="rs")
                nc.vector.reciprocal(out=rs[:], in_=sd[:])
                nc.vector.tensor_scalar(out=dst_ap, in0=src_ap,
                                        scalar1=mv[:, 0:1], scalar2=rs[:],
                                        op0=mybir.AluOpType.subtract,
                                        op1=mybir.AluOpType.mult)
                if gamma_row is not None:
                    nc.vector.tensor_mul(out=dst_ap, in0=dst_ap, in1=gamma_row[:])
                if beta_row is not None:
                    nc.vector.tensor_add(out=dst_ap, in0=dst_ap, in1=beta_row[:])

            # ---------------- front: h0 = LN(gelu(x@W1+b1)) -------------
            for t in range(tiles):
                xt = xpool.tile([128, 2, 128], FP, tag="xt")
                nc.sync.dma_start(
                    out=xt[:],
                    in_=xT[:].rearrange("(k p) n -> p k n", p=128)[:, :, t * 128:(t + 1) * 128])
                ps = ps_f.tile([128, 512], FP, tag="psf")
                for k in range(2):
                    nc.tensor.matmul(out=ps[:, :d], lhsT=xt[:, k, :], rhs=W1t[:, k, :],
                                     start=(k == 0), stop=(k == 1))
                ht = tpool.tile([128, d], FP, tag="ht")
                nc.vector.tensor_add(out=ht[:], in0=ps[:, :d], in1=b1r[:])
                nc.scalar.activation(out=ht[:], in_=ht[:],
                                     func=mybir.ActivationFunctionType.Gelu)
                layernorm(h_sb[:, t, :], ht[:], g1r, be1r)
                nc.scalar.mul(out=ah0_sb[:, t, :], in_=h_sb[:, t, :], mul=ALPHA)
                nc.vector.tensor_scalar_mul(out=g_sb[:, t, :], in0=h_sb[:, t, :],
                                            scalar1=dinv_t[:, t:t + 1])

            def relay_and_allgather(hop, pz):
                t0 = pz * plan.pass_tiles
                t1 = min(t0 + plan.pass_tiles, tiles)
                ai = ag_in[hop % 2]
                lo = t0 * 128 * d
                hi = lo + (t1 - t0) * 128 * d
                nc.sync.dma_start(
                    out=ai[lo:hi].rearrange("(p x) -> p x", p=128),
                    in_=g_sb[:, t0:t1, :])
                glo = pz * N_CORES * plan.pass_tiles * 128 * d
                ghi = glo + N_CORES * (t1 - t0) * 128 * d
                nc.gpsimd.collective_compute(
                    "AllGather", mybir.AluOpType.bypass,
                    ins=[ai[lo:hi]], outs=[g_full[hop % 2][glo:ghi]],
                    replica_groups=rg)

            def relay_hop(hop):
                if AG_SPLIT:
                    for pz in range(plan.n_passes):
                        relay_and_allgather(hop, pz)
                else:
                    ai = ag_in[hop % 2]
                    for pz in range(plan.n_passes):
                        t0 = pz * plan.pass_tiles
                        t1 = min(t0 + plan.pass_tiles, tiles)
                        lo = t0 * 128 * d
                        hi = lo + (t1 - t0) * 128 * d
                        nc.sync.dma_start(
                            out=ai[lo:hi].rearrange("(p x) -> p x", p=128),
                            in_=g_sb[:, t0:t1, :])
                    nc.gpsimd.collective_compute(
                        "AllGather", mybir.AluOpType.bypass,
                        ins=[ai[:]], outs=[g_full[hop % 2][:]],
                        replica_groups=rg)

            relay_hop(0)

            # ---------------- hops ------------------------------------
            for hop in range(1, n_hops + 1):
                gf = g_full[(hop - 1) % 2]
                gf_v = gf[:].rearrange("(r x) -> r x", x=256)
                for pz in range(plan.n_passes):
                    t0 = pz * plan.pass_tiles
                    t1 = min(t0 + plan.pass_tiles, tiles)
                    ntile = t1 - t0
                    nbank = math.ceil(ntile / TILES_PER_BANK)
                    for b in range(nbank):
                        bt0 = t0 + b * TILES_PER_BANK
                        bt1 = min(bt0 + TILES_PER_BANK, t1)
                        bank = ps_s.tile([128, 512], FP, tag="seg",
                                         name=f"seg_{hop}_{pz}_{b}")
                        for tb in range(bt1 - bt0):
                            nc.tensor.matmul(out=bank[:, tb * 64:tb * 64 + 64],
                                             lhsT=ident_bf[:],
                                             rhs=g_sb[:, bt0 + tb, :],
                                             start=(tb == 0), stop=False,
                                             skip_group_check=True)
                        for w in range(2):
                            gk = (pz, b, w)
                            soff, nch = pre["groups"][gk]
                            st = spool.tile([128, nch, 64], BF, tag="smat",
                                            name=f"st_{hop}_{pz}_{b}_{w}",
                                            padded_shape=[128, 96, 64])
                            nc.sync.dma_start(
                                out=st[:],
                                in_=s_d[soff:soff + 128 * nch * 64].rearrange(
                                    "(p x) -> p x", p=128))
                            for (bgk, lo, bc, coff) in pre["blocks"]:
                                if bgk != gk:
                                    continue
                                chunks = pre["grid"][gk][lo:lo + bc]
                                msg = mpool.tile([128, BLOCK_CHUNKS, 128], BF,
                                                 tag="msg")
                                nc.gpsimd.dma_gather(
                                    msg[:, :bc, :], gf_v[:, w * 128:(w + 1) * 128],
                                    idx_t[:, coff:coff + bc * 8],
                                    bc * 128, bc * 128, 128, elem_step=256)
                                for ci, (t, s) in enumerate(chunks):
                                    tb = t - bt0
                                    for par in range(2):
                                        nc.tensor.matmul(
                                            out=bank[s * 32:s * 32 + 32,
                                                     tb * 64:tb * 64 + 64],
                                            lhsT=st[:, lo + ci,
                                                    par * 32:par * 32 + 32],
                                            rhs=msg[:, ci, par * 64:par * 64 + 64],
                                            start=False, stop=False,
                                            skip_group_check=True,
                                            tile_position=(0, s * 32))
                        for tb in range(bt1 - bt0):
                            t = bt0 + tb
                            nc.vector.tensor_scalar_mul(
                                out=h_sb[:, t, :],
                                in0=bank[:, tb * 64:tb * 64 + 64],
                                scalar1=dsc_t[:, t:t + 1])
                            nc.vector.tensor_add(out=h_sb[:, t, :],
                                                 in0=h_sb[:, t, :],
                                                 in1=ah0_sb[:, t, :])
                            nc.vector.tensor_scalar_mul(
                                out=g_sb[:, t, :], in0=h_sb[:, t, :],
                                scalar1=dinv_t[:, t:t + 1])
                    if hop < n_hops and AG_SPLIT:
                        relay_and_allgather(hop, pz)
                if hop < n_hops and not AG_SPLIT:
                    relay_hop(hop)

            # ---------------- back: y = LN(gelu(h)) @ W2p + b2p ----------
            for t in range(tiles):
                gt = tpool.tile([128, d], FP, tag="gt")
                nc.scalar.activation(out=gt[:], in_=h_sb[:, t, :],
                                     func=mybir.ActivationFunctionType.Gelu)
                lt = tpool.tile([128, d], FP, tag="lt")
                layernorm(lt[:], gt[:], None, None)
                pst = ps_f.tile([128, 512], FP, tag="psf")
                nc.tensor.transpose(out=pst[:64, :128], in_=lt[:], identity=ident[:])
                htr = tpool.tile([64, 128], FP, tag="htr")
                nc.vector.tensor_copy(out=htr[:], in_=pst[:64, :128])
                pso = ps_f.tile([128, 512], FP, tag="psf")
                nc.tensor.matmul(out=pso[:, :OUT_C], lhsT=htr[:], rhs=W2t[:],
                                 start=True, stop=True)
                yt = tpool.tile([128, OUT_C], FP, tag="yt")
                nc.vector.tensor_add(out=yt[:], in0=pso[:, :OUT_C], in1=b2r[:])
                nc.sync.dma_start(out=y[t * 128:(t + 1) * 128, :], in_=yt[:])
    nc.finalize()
    return nc


def make_in_maps(pre, W1, b1, g1, be1, g2, be2, W2, b2):
    W2p = (np.asarray(g2)[:, None] * np.asarray(W2)).astype(np.float32)
    b2p = (np.asarray(be2) @ np.asarray(W2) + np.asarray(b2)).astype(np.float32)
    in_maps = []
    for c in range(N_CORES):
        in_maps.append({
            "xT": pre["x_t"][c],
            "W1": np.asarray(W1, np.float32), "b1": np.asarray(b1, np.float32),
            "g1": np.asarray(g1, np.float32), "be1": np.asarray(be1, np.float32),
            "W2p": W2p, "b2p": b2p,
            "dinv": pre["dinv_t"][c], "dsc": pre["dsc_t"][c],
            "idxs": pre["idx_bufs"][c], "smat": pre["s_bufs"][c],
        })
    return in_maps


def kernel(x, edge_index, W1, b1, g1, be1, g2, be2, W2, b2):
    from concourse.bass_utils import run_bass_kernel_spmd
    x = np.asarray(x, np.float32)
    edge_index = np.asarray(edge_index)
    plan = Plan(N, E)
    pre = preprocess(x, edge_index, plan)
    nc = build_nc(plan, pre)
    in_maps = make_in_maps(pre, W1, b1, g1, be1, g2, be2, W2, b2)
    res = run_bass_kernel_spmd(nc, in_maps, core_ids=list(range(N_CORES)),
                               trace=False)
    pc = plan.per_core
    out = np.empty((N, OUT_C), np.float32)
    for c in range(N_CORES):
        out[c * pc:(c + 1) * pc] = res.results[c]["y"][:pc]
    return out



# revision 4
# speedup vs baseline: 1.7569x; 1.7569x over previous
"""APPNP GNN kernel for 8 Trainium2 NeuronCores.

Pipeline: h = gelu(x@W1+b1); h = LN(h,g1,be1); h = APPNP_K10(h); h = gelu(h);
h = LN(h,g2,be2); out = h@W2+b2.

Strategy: destination-sharded nodes (12500/core, padded 12544 = 98 tiles of
128). Per hop, with g = dinv*h folded so the segment-sum is unweighted:
    h'[d] = (1-a)*dinv[d]*(sum_{e->d} g[src_e] + g[d]) + a*h0[d]
g is kept in bf16. Each hop AllGathers the local g shard into a replicated
g_full HBM buffer (partition-contiguous layout), then dma_gather pulls
256-byte PAIRS of g rows per edge (int16 indices address 512B superrows of 4
nodes; 2 residue windows select the superrow half). The segment-sum runs on
the TensorEngine: per 128-edge chunk two banded 0/1 matmuls (even/odd source
parity) accumulate into the PSUM 32-strip of the destination tile. The chunk
grid (tile x window x 32-strip quotas) is computed on the host from ALL
cores' edge counts so the traced program is identical across cores (SPMD).
"""
import math
import numpy as np
import ml_dtypes

import concourse.bass as bass
import concourse.bacc as bacc
import concourse.mybir as mybir
import concourse.tile as tile_mod
from concourse.tile import TileContext
from concourse.masks import make_identity

# ---- workaround: walrus CTRL instructions support few sync-wait slots; the
# TileContext exit drain carries one wait per live semaphore. Split excess
# waits onto individual SP nops emitted before a second drain. ----
_MAX_CTRL_WAITS = 1


def _patched_drain_and_barrier(self, tick_clock, wait_clock):
    drain_inst = self.nc.sync.drain()
    wait_clock.add_sem_waits(
        drain_inst.ins, tile_mod.ScopedClock({None: tick_clock.global_clock}))
    si = drain_inst.ins.sync_info
    if si is not None and si.on_wait and len(si.on_wait) > _MAX_CTRL_WAITS:
        waits = list(si.on_wait)
        si.on_wait = waits[:_MAX_CTRL_WAITS]
        for w in waits[_MAX_CTRL_WAITS:]:
            nop = self.nc.sync.nop(nofuse=True, hint="drain_wait_split")
            nsi = nop.ins.sync_info
            if nsi is None:
                nop.ins.sync_info = type(si)(on_wait=[w], on_update=[])
            else:
                nsi.on_wait = [w]
        self.nc.sync.drain()
    self.nc.all_engine_barrier()
    assert self.sems is not None
    popped = self.nc._tile_sem_poison_stack.pop()
    assert popped is self._sem_poison
    self.nc.clear_and_free_semaphores(list(self.sems.allocated().values()))
    self.nc.all_engine_barrier()


TileContext._drain_and_barrier = _patched_drain_and_barrier

FP = mybir.dt.float32
BF = mybir.dt.bfloat16

N, E, IN_C, HID, OUT_C = 100000, 1600000, 256, 64, 64
K_HOPS, ALPHA, LN_EPS = 10, 0.1, 1e-5
N_CORES = 8

TILES_PER_BANK = 7
BANKS = 7          # PSUM banks used per segsum pass (1 left for front/back)
BLOCK_CHUNKS = 8   # chunks per gather call (SWDGE ring caps at 1024 desc/dir)
GATHER_QUEUES = 4  # SWDGE queues round-robin: each queue ~7.5ns/desc serial,
                   # 4 queues measured 1.21ns/desc (micro-bench, data-checked)
AG_SPLIT = False   # one AllGather per hop (ncfw per-collective fixed cost beats split overlap on HW)


def pack_idx16(idx: np.ndarray) -> np.ndarray:
    """[n] -> [128, n/16] int16 (16-partition wrap, replicated 8x for Q7s)."""
    n = idx.shape[0]
    assert n % 16 == 0
    t = idx.astype(np.int16).reshape(n // 16, 16).T
    return np.tile(t, (8, 1))


class Plan:
    def __init__(self, n_total, e_total):
        self.n_total = n_total
        self.per_core = n_total // N_CORES
        self.sh = ((self.per_core + 127) // 128) * 128
        self.tiles = self.sh // 128
        self.pass_tiles = BANKS * TILES_PER_BANK
        self.n_passes = math.ceil(self.tiles / self.pass_tiles)
        self.rows_full = self.sh * N_CORES
        assert self.rows_full % 4 == 0
        assert (self.rows_full // 4) - 1 <= 32767, "int16 idx overflow"


def preprocess(x, edge_index, plan):
    pc, sh, tiles = plan.per_core, plan.sh, plan.tiles
    src = edge_index[0].astype(np.int64)
    dst = edge_index[1].astype(np.int64)

    deg = np.bincount(dst, minlength=plan.n_total).astype(np.float64) + 1.0
    dinv = (1.0 / np.sqrt(deg)).astype(np.float32)

    c_dst = dst // pc
    ld = dst - c_dst * pc
    e_t, e_p = ld // 128, ld % 128
    e_s = e_p // 32
    c_src = src // pc
    rs = src - c_src * pc
    s_t, s_p = rs // 128, rs % 128
    s_pz = s_t // plan.pass_tiles
    s_tl = s_t - s_pz * plan.pass_tiles
    tp = np.minimum(plan.pass_tiles, tiles - s_pz * plan.pass_tiles)
    gpos = (c_src * sh + s_pz * plan.pass_tiles * 128
            + s_p * tp + s_tl)
    e_w = ((gpos % 4) // 2).astype(np.int64)
    e_par = (gpos % 2).astype(np.int64)
    e_idx = (gpos // 4).astype(np.int16)

    # shared chunk quotas per (tile, window, strip)
    cnt = np.zeros((N_CORES, tiles, 2, 4), np.int64)
    np.add.at(cnt.reshape(-1), ((c_dst * tiles + e_t) * 2 + e_w) * 4 + e_s, 1)
    quota = np.maximum(1, np.ceil(cnt.max(axis=0) / 128.0).astype(np.int64))

    grid = {}  # (pass, bank, w) -> ordered [(tile, strip)] chunk list
    for pz in range(plan.n_passes):
        t0 = pz * plan.pass_tiles
        t1 = min(t0 + plan.pass_tiles, tiles)
        nbank = math.ceil((t1 - t0) / TILES_PER_BANK)
        for b in range(nbank):
            bt0 = t0 + b * TILES_PER_BANK
            bt1 = min(bt0 + TILES_PER_BANK, t1)
            for w in range(2):
                chunks = []
                for t in range(bt0, bt1):
                    for s in range(4):
                        chunks.extend([(t, s)] * int(quota[t, w, s]))
                grid[(pz, b, w)] = chunks

    order = np.lexsort((e_p, e_s, e_w, e_t, c_dst))
    so_idx = e_idx[order]
    so_par = e_par[order].astype(np.int64)
    so_p = e_p[order].astype(np.int64)
    gkey = (((c_dst[order] * tiles + e_t[order]) * 2 + e_w[order]) * 4 + e_s[order])
    n_keys = N_CORES * tiles * 2 * 4
    starts = np.searchsorted(gkey, np.arange(n_keys))
    ends = np.searchsorted(gkey, np.arange(n_keys), side="right")

    # block structure (shared): gather blocks within each (pass,bank,w) group;
    # one S-load per group
    gkeys = sorted(grid.keys())
    blocks = []   # (gkey, chunk_lo, n_chunks, idx_col_off)
    groups = {}   # gkey -> (s_off, nch)
    idx_cols_total, s_size = 0, 0
    for gk in gkeys:
        nch = len(grid[gk])
        groups[gk] = (s_size, nch)
        s_size += 128 * nch * 64
        for lo in range(0, nch, BLOCK_CHUNKS):
            bc = min(BLOCK_CHUNKS, nch - lo)
            blocks.append((gk, lo, bc, idx_cols_total))
            idx_cols_total += bc * 8

    idx_bufs, s_bufs = [], []
    for c in range(N_CORES):
        idx_buf = np.zeros((128, idx_cols_total), np.int16)
        s_flat = np.zeros(s_size, ml_dtypes.bfloat16)
        for gk in gkeys:
            pz, b, w = gk
            chunks = grid[gk]
            nch = len(chunks)
            soff, _ = groups[gk]
            slots = np.zeros((nch, 128), np.int16)
            sm = np.zeros((nch, 128, 64), ml_dtypes.bfloat16)
            ci = 0
            while ci < nch:
                t, s = chunks[ci]
                reps = 1
                while ci + reps < nch and chunks[ci + reps] == (t, s):
                    reps += 1
                key = ((c * tiles + t) * 2 + w) * 4 + s
                s0, s1 = int(starts[key]), int(ends[key])
                npz = s1 - s0
                assert npz <= reps * 128
                if npz:
                    ce = ci + np.arange(npz) // 128        # chunk id
                    sl = np.arange(npz) % 128              # slot in chunk
                    slots[ce, sl] = so_idx[s0:s1]
                    col = (so_p[s0:s1] % 32) + 32 * so_par[s0:s1]
                    sm[ce, sl, col] = 1.0
                ci += reps
            s_flat[soff:soff + 128 * nch * 64] = (
                sm.transpose(1, 0, 2).reshape(-1))
            for (bgk, lo, bc, coff) in blocks:
                if bgk != gk:
                    continue
                idx_buf[:, coff:coff + bc * 8] = pack_idx16(
                    slots[lo:lo + bc].reshape(-1))
        idx_bufs.append(idx_buf)
        s_bufs.append(s_flat)

    dinv_t, dsc_t, x_t = [], [], []
    for c in range(N_CORES):
        dv = np.ones(sh, np.float32)
        dv[:pc] = dinv[c * pc:(c + 1) * pc]
        dinv_t.append(dv.reshape(tiles, 128).T.copy())
        dsc_t.append(((1.0 - ALPHA) * dv).reshape(tiles, 128).T.copy())
        xs = np.zeros((sh, IN_C), np.float32)
        xs[:pc] = x[c * pc:(c + 1) * pc]
        x_t.append(xs.T.copy())
    return {
        "grid": grid, "blocks": blocks, "groups": groups,
        "idx_cols_total": idx_cols_total,
        "s_size": s_size, "idx_bufs": idx_bufs, "s_bufs": s_bufs,
        "dinv_t": dinv_t, "dsc_t": dsc_t, "x_t": x_t,
    }


def build_nc(plan, pre, n_hops=K_HOPS):
    gq = [0]
    nc = bacc.Bacc("TRN2", num_devices=N_CORES, num_swdge_queues=GATHER_QUEUES)
    sh, tiles = plan.sh, plan.tiles
    d = HID

    xT = nc.dram_tensor("xT", [IN_C, sh], FP, kind="ExternalInput")
    W1 = nc.dram_tensor("W1", [IN_C, HID], FP, kind="ExternalInput")
    b1 = nc.dram_tensor("b1", [HID], FP, kind="ExternalInput")
    g1 = nc.dram_tensor("g1", [HID], FP, kind="ExternalInput")
    be1 = nc.dram_tensor("be1", [HID], FP, kind="ExternalInput")
    W2p = nc.dram_tensor("W2p", [HID, OUT_C], FP, kind="ExternalInput")
    b2p = nc.dram_tensor("b2p", [OUT_C], FP, kind="ExternalInput")
    dinv_d = nc.dram_tensor("dinv", [128, tiles], FP, kind="ExternalInput")
    dsc_d = nc.dram_tensor("dsc", [128, tiles], FP, kind="ExternalInput")
    idx_d = nc.dram_tensor("idxs", [128, pre["idx_cols_total"]], mybir.dt.int16,
                           kind="ExternalInput")
    s_d = nc.dram_tensor("smat", [pre["s_size"]], BF, kind="ExternalInput")
    y = nc.dram_tensor("y", [sh, OUT_C], FP, kind="ExternalOutput")

    ag_in = [nc.dram_tensor(f"ag_in{i}", [sh * d], BF) for i in range(2)]
    g_full = [nc.dram_tensor(f"g_full{i}", [plan.rows_full * d], BF,
                             addr_space="Shared") for i in range(2)]
    rg = [list(range(N_CORES))]

    def bcast_row(pool, dram, width):
        tile = pool.tile([128, width], FP, tag=f"bc_{dram.name}",
                         name=f"bc_{dram.name}")
        ap = bass.AP(tensor=dram, offset=0, ap=[[0, 128], [1, width]])
        nc.gpsimd.dma_start(out=tile[:], in_=ap)
        return tile

    with TileContext(nc) as tc:
        import contextlib
        with contextlib.ExitStack() as ctx:
            const = ctx.enter_context(tc.tile_pool(name="const", bufs=1))
            mpool = ctx.enter_context(tc.tile_pool(name="msg", bufs=8))
            spool = ctx.enter_context(tc.tile_pool(name="smat", bufs=3))
            xpool = ctx.enter_context(tc.tile_pool(name="xt", bufs=3))
            tpool = ctx.enter_context(tc.tile_pool(name="tmp", bufs=6))
            ps_f = ctx.enter_context(tc.tile_pool(name="psf", bufs=1, space="PSUM"))
            ps_s = ctx.enter_context(tc.tile_pool(name="pss", bufs=BANKS, space="PSUM"))

            ident = const.tile([128, 128], FP, tag="ident")
            make_identity(nc, ident[:])
            ident_bf = const.tile([128, 128], BF, tag="identbf")
            nc.vector.tensor_copy(out=ident_bf[:], in_=ident[:])
            eps_t = const.tile([128, 1], FP, tag="eps")
            nc.vector.memset(eps_t[:], LN_EPS)
            b1r = bcast_row(const, b1, HID)
            g1r = bcast_row(const, g1, HID)
            be1r = bcast_row(const, be1, HID)
            b2r = bcast_row(const, b2p, OUT_C)
            W1t = const.tile([128, 2, HID], FP, tag="w1")
            nc.sync.dma_start(out=W1t[:], in_=W1[:].rearrange("(k p) d -> p k d", p=128))
            W2t = const.tile([64, OUT_C], FP, tag="w2")
            nc.sync.dma_start(out=W2t[:], in_=W2p[:])
            dinv_t = const.tile([128, tiles], FP, tag="dinv")
            nc.sync.dma_start(out=dinv_t[:], in_=dinv_d[:])
            dsc_t = const.tile([128, tiles], FP, tag="dsc")
            nc.sync.dma_start(out=dsc_t[:], in_=dsc_d[:])
            idx_t = const.tile([128, pre["idx_cols_total"]], mybir.dt.int16, tag="idx")
            nc.sync.dma_start(out=idx_t[:], in_=idx_d[:])

            h_sb = const.tile([128, tiles, d], FP, tag="h")
            ah0_sb = const.tile([128, tiles, d], FP, tag="ah0")
            g_sb = const.tile([128, tiles, d], BF, tag="g")

            def layernorm(dst_ap, src_ap, gamma_row, beta_row):
                stats = tpool.tile([128, 6], FP, tag="stats", name="stats")
                mv = tpool.tile([128, 2], FP, tag="mv", name="mv")
                nc.vector.bn_stats(out=stats[:], in_=src_ap)
                nc.vector.bn_aggr(out=mv[:], in_=stats[:])
                sd = tpool.tile([128, 1], FP, tag="sd", name="sd")
                nc.scalar.activation(out=sd[:], in_=mv[:, 1:2],
                                     func=mybir.ActivationFunctionType.Sqrt,
                                     bias=eps_t[:], scale=1.0)
                rs = tpool.tile([128, 1], FP, tag="rs", name="rs")
                nc.vector.reciprocal(out=rs[:], in_=sd[:])
                nc.vector.tensor_scalar(out=dst_ap, in0=src_ap,
                                        scalar1=mv[:, 0:1], scalar2=rs[:],
                                        op0=mybir.AluOpType.subtract,
                                        op1=mybir.AluOpType.mult)
                if gamma_row is not None:
                    nc.vector.tensor_mul(out=dst_ap, in0=dst_ap, in1=gamma_row[:])
                if beta_row is not None:
                    nc.vector.tensor_add(out=dst_ap, in0=dst_ap, in1=beta_row[:])

            # ---------------- front: h0 = LN(gelu(x@W1+b1)) -------------
            for t in range(tiles):
                xt = xpool.tile([128, 2, 128], FP, tag="xt")
                nc.sync.dma_start(
                    out=xt[:],
                    in_=xT[:].rearrange("(k p) n -> p k n", p=128)[:, :, t * 128:(t + 1) * 128])
                ps = ps_f.tile([128, 512], FP, tag="psf")
                for k in range(2):
                    nc.tensor.matmul(out=ps[:, :d], lhsT=xt[:, k, :], rhs=W1t[:, k, :],
                                     start=(k == 0), stop=(k == 1))
                ht = tpool.tile([128, d], FP, tag="ht")
                nc.vector.tensor_add(out=ht[:], in0=ps[:, :d], in1=b1r[:])
                nc.scalar.activation(out=ht[:], in_=ht[:],
                                     func=mybir.ActivationFunctionType.Gelu)
                layernorm(h_sb[:, t, :], ht[:], g1r, be1r)
                nc.scalar.mul(out=ah0_sb[:, t, :], in_=h_sb[:, t, :], mul=ALPHA)
                nc.vector.tensor_scalar_mul(out=g_sb[:, t, :], in0=h_sb[:, t, :],
                                            scalar1=dinv_t[:, t:t + 1])

            def relay_and_allgather(hop, pz):
                t0 = pz * plan.pass_tiles
                t1 = min(t0 + plan.pass_tiles, tiles)
                ai = ag_in[hop % 2]
                lo = t0 * 128 * d
                hi = lo + (t1 - t0) * 128 * d
                nc.sync.dma_start(
                    out=ai[lo:hi].rearrange("(p x) -> p x", p=128),
                    in_=g_sb[:, t0:t1, :])
                glo = pz * N_CORES * plan.pass_tiles * 128 * d
                ghi = glo + N_CORES * (t1 - t0) * 128 * d
                nc.gpsimd.collective_compute(
                    "AllGather", mybir.AluOpType.bypass,
                    ins=[ai[lo:hi]], outs=[g_full[hop % 2][glo:ghi]],
                    replica_groups=rg)

            def relay_hop(hop):
                if AG_SPLIT:
                    for pz in range(plan.n_passes):
                        relay_and_allgather(hop, pz)
                else:
                    ai = ag_in[hop % 2]
                    for pz in range(plan.n_passes):
                        t0 = pz * plan.pass_tiles
                        t1 = min(t0 + plan.pass_tiles, tiles)
                        lo = t0 * 128 * d
                        hi = lo + (t1 - t0) * 128 * d
                        nc.sync.dma_start(
                            out=ai[lo:hi].rearrange("(p x) -> p x", p=128),
                            in_=g_sb[:, t0:t1, :])
                    nc.gpsimd.collective_compute(
                        "AllGather", mybir.AluOpType.bypass,
                        ins=[ai[:]], outs=[g_full[hop % 2][:]],
                        replica_groups=rg)

            relay_hop(0)

            # ---------------- hops ------------------------------------
            for hop in range(1, n_hops + 1):
                gf = g_full[(hop - 1) % 2]
                gf_v = gf[:].rearrange("(r x) -> r x", x=256)
                for pz in range(plan.n_passes):
                    t0 = pz * plan.pass_tiles
                    t1 = min(t0 + plan.pass_tiles, tiles)
                    ntile = t1 - t0
                    nbank = math.ceil(ntile / TILES_PER_BANK)
                    for b in range(nbank):
                        bt0 = t0 + b * TILES_PER_BANK
                        bt1 = min(bt0 + TILES_PER_BANK, t1)
                        bank = ps_s.tile([128, 512], FP, tag="seg",
                                         name=f"seg_{hop}_{pz}_{b}")
                        for tb in range(bt1 - bt0):
                            nc.tensor.matmul(out=bank[:, tb * 64:tb * 64 + 64],
                                             lhsT=ident_bf[:],
                                             rhs=g_sb[:, bt0 + tb, :],
                                             start=(tb == 0), stop=False,
                                             skip_group_check=True)
                        for w in range(2):
                            gk = (pz, b, w)
                            soff, nch = pre["groups"][gk]
                            st = spool.tile([128, nch, 64], BF, tag="smat",
                                            name=f"st_{hop}_{pz}_{b}_{w}",
                                            padded_shape=[128, 96, 64])
                            nc.sync.dma_start(
                                out=st[:],
                                in_=s_d[soff:soff + 128 * nch * 64].rearrange(
                                    "(p x) -> p x", p=128))
                            for (bgk, lo, bc, coff) in pre["blocks"]:
                                if bgk != gk:
                                    continue
                                chunks = pre["grid"][gk][lo:lo + bc]
                                msg = mpool.tile([128, BLOCK_CHUNKS, 128], BF,
                                                 tag="msg")
                                nc.gpsimd.dma_gather(
                                    msg[:, :bc, :], gf_v[:, w * 128:(w + 1) * 128],
                                    idx_t[:, coff:coff + bc * 8],
                                    bc * 128, bc * 128, 128, elem_step=256,
                                    queue_num=gq[0] % GATHER_QUEUES)
                                gq[0] += 1
                                for ci, (t, s) in enumerate(chunks):
                                    tb = t - bt0
                                    for par in range(2):
                                        nc.tensor.matmul(
                                            out=bank[s * 32:s * 32 + 32,
                                                     tb * 64:tb * 64 + 64],
                                            lhsT=st[:, lo + ci,
                                                    par * 32:par * 32 + 32],
                                            rhs=msg[:, ci, par * 64:par * 64 + 64],
                                            start=False, stop=False,
                                            skip_group_check=True,
                                            tile_position=(0, s * 32))
                        for tb in range(bt1 - bt0):
                            t = bt0 + tb
                            nc.vector.tensor_scalar_mul(
                                out=h_sb[:, t, :],
                                in0=bank[:, tb * 64:tb * 64 + 64],
                                scalar1=dsc_t[:, t:t + 1])
                            nc.vector.tensor_add(out=h_sb[:, t, :],
                                                 in0=h_sb[:, t, :],
                                                 in1=ah0_sb[:, t, :])
                            nc.vector.tensor_scalar_mul(
                                out=g_sb[:, t, :], in0=h_sb[:, t, :],
                                scalar1=dinv_t[:, t:t + 1])
                    if hop < n_hops and AG_SPLIT:
                        relay_and_allgather(hop, pz)
                if hop < n_hops and not AG_SPLIT:
                    relay_hop(hop)

            # ---------------- back: y = LN(gelu(h)) @ W2p + b2p ----------
            for t in range(tiles):
                gt = tpool.tile([128, d], FP, tag="gt")
                nc.scalar.activation(out=gt[:], in_=h_sb[:, t, :],
                                     func=mybir.ActivationFunctionType.Gelu)
                lt = tpool.tile([128, d], FP, tag="lt")
                layernorm(lt[:], gt[:], None, None)
                pst = ps_f.tile([128, 512], FP, tag="psf")
                nc.tensor.transpose(out=pst[:64, :128], in_=lt[:], identity=ident[:])
                htr = tpool.tile([64, 128], FP, tag="htr")
                nc.vector.tensor_copy(out=htr[:], in_=pst[:64, :128])
                pso = ps_f.tile([128, 512], FP, tag="psf")
                nc.tensor.matmul(out=pso[:, :OUT_C], lhsT=htr[:], rhs=W2t[:],
                                 start=True, stop=True)
                yt = tpool.tile([128, OUT_C], FP, tag="yt")
                nc.vector.tensor_add(out=yt[:], in0=pso[:, :OUT_C], in1=b2r[:])
                nc.sync.dma_start(out=y[t * 128:(t + 1) * 128, :], in_=yt[:])
    nc.finalize()
    return nc


def make_in_maps(pre, W1, b1, g1, be1, g2, be2, W2, b2):
    W2p = (np.asarray(g2)[:, None] * np.asarray(W2)).astype(np.float32)
    b2p = (np.asarray(be2) @ np.asarray(W2) + np.asarray(b2)).astype(np.float32)
    in_maps = []
    for c in range(N_CORES):
        in_maps.append({
            "xT": pre["x_t"][c],
            "W1": np.asarray(W1, np.float32), "b1": np.asarray(b1, np.float32),
            "g1": np.asarray(g1, np.float32), "be1": np.asarray(be1, np.float32),
            "W2p": W2p, "b2p": b2p,
            "dinv": pre["dinv_t"][c], "dsc": pre["dsc_t"][c],
            "idxs": pre["idx_bufs"][c], "smat": pre["s_bufs"][c],
        })
    return in_maps


def kernel(x, edge_index, W1, b1, g1, be1, g2, be2, W2, b2):
    from concourse.bass_utils import run_bass_kernel_spmd
    x = np.asarray(x, np.float32)
    edge_index = np.asarray(edge_index)
    plan = Plan(N, E)
    pre = preprocess(x, edge_index, plan)
    nc = build_nc(plan, pre)
    in_maps = make_in_maps(pre, W1, b1, g1, be1, g2, be2, W2, b2)
    res = run_bass_kernel_spmd(nc, in_maps, core_ids=list(range(N_CORES)),
                               trace=False)
    pc = plan.per_core
    out = np.empty((N, OUT_C), np.float32)
    for c in range(N_CORES):
        out[c * pc:(c + 1) * pc] = res.results[c]["y"][:pc]
    return out



# revision 6
# speedup vs baseline: 1.7879x; 1.0177x over previous
"""APPNP GNN kernel v2 for 8 Trainium2 NeuronCores.

Pipeline: h = gelu(x@W1+b1); h = LN(h,g1,be1); h = APPNP_K10(h); h = gelu(h);
h = LN(h,g2,be2); out = h@W2+b2.

v2 vs v1: 4 SWDGE gather queues (measured 1.21 ns/desc vs 7.5 at q1);
bins coarsened from (tile,window,32-strip) to (tile,window) — full-tile
128-wide stationary matmuls — cutting descriptors 299k -> 225k per hop; the
selection matrices are generated ON DEVICE per chunk with a single DVE
is_equal against an iota row (S01[p,m] = (iota256[m] == dstrow[p]+128*par[p]))
instead of streaming 38MB/hop of precomputed one-hot matrices from HBM.
Per hop, per chunk of 128 edges: one [128,256] S-gen + two accumulating
matmuls (parity halves of the gathered 256B pair rows).
"""
import math
import numpy as np
import ml_dtypes

import concourse.bass as bass
import concourse.bacc as bacc
import concourse.mybir as mybir
import concourse.tile as tile_mod
from concourse.tile import TileContext
from concourse.masks import make_identity

# ---- workaround: walrus CTRL instructions support few sync-wait slots; the
# TileContext exit drain carries one wait per live semaphore. Split excess
# waits onto individual SP nops emitted before a second drain. ----
_MAX_CTRL_WAITS = 1


def _patched_drain_and_barrier(self, tick_clock, wait_clock):
    drain_inst = self.nc.sync.drain()
    wait_clock.add_sem_waits(
        drain_inst.ins, tile_mod.ScopedClock({None: tick_clock.global_clock}))
    si = drain_inst.ins.sync_info
    if si is not None and si.on_wait and len(si.on_wait) > _MAX_CTRL_WAITS:
        waits = list(si.on_wait)
        si.on_wait = waits[:_MAX_CTRL_WAITS]
        for w in waits[_MAX_CTRL_WAITS:]:
            nop = self.nc.sync.nop(nofuse=True, hint="drain_wait_split")
            nsi = nop.ins.sync_info
            if nsi is None:
                nop.ins.sync_info = type(si)(on_wait=[w], on_update=[])
            else:
                nsi.on_wait = [w]
        self.nc.sync.drain()
    self.nc.all_engine_barrier()
    assert self.sems is not None
    popped = self.nc._tile_sem_poison_stack.pop()
    assert popped is self._sem_poison
    self.nc.clear_and_free_semaphores(list(self.sems.allocated().values()))
    self.nc.all_engine_barrier()


TileContext._drain_and_barrier = _patched_drain_and_barrier

FP = mybir.dt.float32
BF = mybir.dt.bfloat16

N, E, IN_C, HID, OUT_C = 100000, 1600000, 256, 64, 64
K_HOPS, ALPHA, LN_EPS = 10, 0.1, 1e-5
N_CORES = 8

TILES_PER_BANK = 7
BANKS = 7          # PSUM banks used per segsum pass (1 left for front/back)
BLOCK_CHUNKS = 8   # chunks per gather call (SWDGE ring caps at 1024 desc/dir)
GATHER_QUEUES = 4
PAD_DR = 384.0     # dstrow01 value for empty slots (matches no iota column)


def pack_idx16(idx: np.ndarray) -> np.ndarray:
    """[n] -> [128, n/16] int16 (16-partition wrap, replicated 8x for Q7s)."""
    n = idx.shape[0]
    assert n % 16 == 0
    t = idx.astype(np.int16).reshape(n // 16, 16).T
    return np.tile(t, (8, 1))


class Plan:
    def __init__(self, n_total, e_total):
        self.n_total = n_total
        self.per_core = n_total // N_CORES
        self.sh = ((self.per_core + 127) // 128) * 128
        self.tiles = self.sh // 128
        self.pass_tiles = BANKS * TILES_PER_BANK
        self.n_passes = math.ceil(self.tiles / self.pass_tiles)
        self.rows_full = self.sh * N_CORES
        assert self.rows_full % 4 == 0
        assert (self.rows_full // 4) - 1 <= 32767, "int16 idx overflow"


def preprocess(x, edge_index, plan):
    pc, sh, tiles = plan.per_core, plan.sh, plan.tiles
    src = edge_index[0].astype(np.int64)
    dst = edge_index[1].astype(np.int64)

    deg = np.bincount(dst, minlength=plan.n_total).astype(np.float64) + 1.0
    dinv = (1.0 / np.sqrt(deg)).astype(np.float32)

    c_dst = dst // pc
    ld = dst - c_dst * pc
    e_t, e_p = ld // 128, ld % 128
    c_src = src // pc
    rs = src - c_src * pc
    s_t, s_p = rs // 128, rs % 128
    s_pz = s_t // plan.pass_tiles
    s_tl = s_t - s_pz * plan.pass_tiles
    tp = np.minimum(plan.pass_tiles, tiles - s_pz * plan.pass_tiles)
    gpos = (c_src * sh + s_pz * plan.pass_tiles * 128
            + s_p * tp + s_tl)
    e_w = ((gpos % 4) // 2).astype(np.int64)
    e_par = (gpos % 2).astype(np.int64)
    e_idx = (gpos // 4).astype(np.int16)

    # shared chunk quotas per (tile, window)
    cnt = np.zeros((N_CORES, tiles, 2), np.int64)
    np.add.at(cnt.reshape(-1), (c_dst * tiles + e_t) * 2 + e_w, 1)
    quota = np.maximum(1, np.ceil(cnt.max(axis=0) / 128.0).astype(np.int64))

    grid = {}  # (pass, bank, w) -> ordered [tile] chunk list
    for pz in range(plan.n_passes):
        t0 = pz * plan.pass_tiles
        t1 = min(t0 + plan.pass_tiles, tiles)
        nbank = math.ceil((t1 - t0) / TILES_PER_BANK)
        for b in range(nbank):
            bt0 = t0 + b * TILES_PER_BANK
            bt1 = min(bt0 + TILES_PER_BANK, t1)
            for w in range(2):
                chunks = []
                for t in range(bt0, bt1):
                    chunks.extend([t] * int(quota[t, w]))
                grid[(pz, b, w)] = chunks

    order = np.lexsort((e_par, e_p, e_w, e_t, c_dst))
    so_idx = e_idx[order]
    so_par = e_par[order].astype(np.int64)
    so_p = e_p[order].astype(np.int64)
    gkey = (c_dst[order] * tiles + e_t[order]) * 2 + e_w[order]
    n_keys = N_CORES * tiles * 2
    starts = np.searchsorted(gkey, np.arange(n_keys))
    ends = np.searchsorted(gkey, np.arange(n_keys), side="right")

    # block structure (shared): gather blocks within each (pass,bank,w) group
    gkeys = sorted(grid.keys())
    blocks = []   # (gkey, chunk_lo, n_chunks, idx_col_off)
    chunk_off = {}  # gkey -> first global chunk index
    idx_cols_total, n_chunks_total = 0, 0
    for gk in gkeys:
        nch = len(grid[gk])
        chunk_off[gk] = n_chunks_total
        n_chunks_total += nch
        for lo in range(0, nch, BLOCK_CHUNKS):
            bc = min(BLOCK_CHUNKS, nch - lo)
            blocks.append((gk, lo, bc, idx_cols_total))
            idx_cols_total += bc * 8

    idx_bufs, dr_bufs = [], []
    for c in range(N_CORES):
        idx_buf = np.zeros((128, idx_cols_total), np.int16)
        dr_buf = np.full((n_chunks_total, 128), PAD_DR, np.float32)
        for gk in gkeys:
            pz, b, w = gk
            chunks = grid[gk]
            nch = len(chunks)
            coff0 = chunk_off[gk]
            slots = np.zeros((nch, 128), np.int16)
            ci = 0
            while ci < nch:
                t = chunks[ci]
                reps = 1
                while ci + reps < nch and chunks[ci + reps] == t:
                    reps += 1
                key = (c * plan.tiles + t) * 2 + w
                s0, s1 = int(starts[key]), int(ends[key])
                npz = s1 - s0
                assert npz <= reps * 128
                if npz:
                    ce = ci + np.arange(npz) // 128        # chunk id
                    sl = np.arange(npz) % 128              # slot in chunk
                    slots[ce, sl] = so_idx[s0:s1]
                    dr_buf[coff0 + ce, sl] = (so_p[s0:s1]
                                              + 128.0 * so_par[s0:s1])
                ci += reps
            for (bgk, lo, bc, coff) in blocks:
                if bgk != gk:
                    continue
                idx_buf[:, coff:coff + bc * 8] = pack_idx16(
                    slots[lo:lo + bc].reshape(-1))
        idx_bufs.append(idx_buf)
        dr_bufs.append(dr_buf.T.copy())  # [128, n_chunks] f32

    dinv_t, dsc_t, x_t = [], [], []
    for c in range(N_CORES):
        dv = np.ones(sh, np.float32)
        dv[:pc] = dinv[c * pc:(c + 1) * pc]
        dinv_t.append(dv.reshape(tiles, 128).T.copy())
        dsc_t.append(((1.0 - ALPHA) * dv).reshape(tiles, 128).T.copy())
        xs = np.zeros((sh, IN_C), np.float32)
        xs[:pc] = x[c * pc:(c + 1) * pc]
        x_t.append(xs.T.copy())
    return {
        "grid": grid, "blocks": blocks, "chunk_off": chunk_off,
        "idx_cols_total": idx_cols_total, "n_chunks_total": n_chunks_total,
        "idx_bufs": idx_bufs, "dr_bufs": dr_bufs,
        "dinv_t": dinv_t, "dsc_t": dsc_t, "x_t": x_t,
    }


def build_nc(plan, pre, n_hops=K_HOPS):
    gq = [0]
    nc = bacc.Bacc("TRN2", num_devices=N_CORES, num_swdge_queues=GATHER_QUEUES)
    sh, tiles = plan.sh, plan.tiles
    d = HID

    xT = nc.dram_tensor("xT", [IN_C, sh], FP, kind="ExternalInput")
    W1 = nc.dram_tensor("W1", [IN_C, HID], FP, kind="ExternalInput")
    b1 = nc.dram_tensor("b1", [HID], FP, kind="ExternalInput")
    g1 = nc.dram_tensor("g1", [HID], FP, kind="ExternalInput")
    be1 = nc.dram_tensor("be1", [HID], FP, kind="ExternalInput")
    W2p = nc.dram_tensor("W2p", [HID, OUT_C], FP, kind="ExternalInput")
    b2p = nc.dram_tensor("b2p", [OUT_C], FP, kind="ExternalInput")
    dinv_d = nc.dram_tensor("dinv", [128, tiles], FP, kind="ExternalInput")
    dsc_d = nc.dram_tensor("dsc", [128, tiles], FP, kind="ExternalInput")
    idx_d = nc.dram_tensor("idxs", [128, pre["idx_cols_total"]], mybir.dt.int16,
                           kind="ExternalInput")
    dr_d = nc.dram_tensor("drow", [128, pre["n_chunks_total"]], FP,
                          kind="ExternalInput")
    iota_d = nc.dram_tensor("iota", [256], FP, kind="ExternalInput")
    y = nc.dram_tensor("y", [sh, OUT_C], FP, kind="ExternalOutput")

    ag_in = [nc.dram_tensor(f"ag_in{i}", [sh * d], BF) for i in range(2)]
    # Local (per-core) AllGather output: gathers from Shared scratchpad were
    # measured ~6x slower per descriptor than from local HBM.
    g_full = [nc.dram_tensor(f"g_full{i}", [plan.rows_full * d], BF)
              for i in range(2)]
    rg = [list(range(N_CORES))]

    def bcast_row(pool, dram, width):
        tile = pool.tile([128, width], FP, tag=f"bc_{dram.name}",
                         name=f"bc_{dram.name}")
        ap = bass.AP(tensor=dram, offset=0, ap=[[0, 128], [1, width]])
        nc.gpsimd.dma_start(out=tile[:], in_=ap)
        return tile

    with TileContext(nc) as tc:
        import contextlib
        with contextlib.ExitStack() as ctx:
            const = ctx.enter_context(tc.tile_pool(name="const", bufs=1))
            mpool = ctx.enter_context(tc.tile_pool(name="msg", bufs=10))
            sgpool = ctx.enter_context(tc.tile_pool(name="sgen", bufs=12))
            xpool = ctx.enter_context(tc.tile_pool(name="xt", bufs=3))
            tpool = ctx.enter_context(tc.tile_pool(name="tmp", bufs=6))
            ps_f = ctx.enter_context(tc.tile_pool(name="psf", bufs=1, space="PSUM"))
            ps_s = ctx.enter_context(tc.tile_pool(name="pss", bufs=BANKS, space="PSUM"))

            ident = const.tile([128, 128], FP, tag="ident")
            make_identity(nc, ident[:])
            ident_bf = const.tile([128, 128], BF, tag="identbf")
            nc.vector.tensor_copy(out=ident_bf[:], in_=ident[:])
            eps_t = const.tile([128, 1], FP, tag="eps")
            nc.vector.memset(eps_t[:], LN_EPS)
            b1r = bcast_row(const, b1, HID)
            g1r = bcast_row(const, g1, HID)
            be1r = bcast_row(const, be1, HID)
            b2r = bcast_row(const, b2p, OUT_C)
            iota_f = bcast_row(const, iota_d, 256)
            iota_bf = const.tile([128, 256], BF, tag="iotabf")
            nc.vector.tensor_copy(out=iota_bf[:], in_=iota_f[:])
            W1t = const.tile([128, 2, HID], FP, tag="w1")
            nc.sync.dma_start(out=W1t[:], in_=W1[:].rearrange("(k p) d -> p k d", p=128))
            W2t = const.tile([64, OUT_C], FP, tag="w2")
            nc.sync.dma_start(out=W2t[:], in_=W2p[:])
            dinv_t = const.tile([128, tiles], FP, tag="dinv")
            nc.sync.dma_start(out=dinv_t[:], in_=dinv_d[:])
            dsc_t = const.tile([128, tiles], FP, tag="dsc")
            nc.sync.dma_start(out=dsc_t[:], in_=dsc_d[:])
            idx_t = const.tile([128, pre["idx_cols_total"]], mybir.dt.int16, tag="idx")
            nc.sync.dma_start(out=idx_t[:], in_=idx_d[:])
            dr_t = const.tile([128, pre["n_chunks_total"]], FP, tag="drow")
            nc.sync.dma_start(out=dr_t[:], in_=dr_d[:])

            h_sb = const.tile([128, tiles, d], FP, tag="h")
            ah0_sb = const.tile([128, tiles, d], FP, tag="ah0")
            g_sb = const.tile([128, tiles, d], BF, tag="g")

            def layernorm(dst_ap, src_ap, gamma_row, beta_row):
                stats = tpool.tile([128, 6], FP, tag="stats", name="stats")
                mv = tpool.tile([128, 2], FP, tag="mv", name="mv")
                nc.vector.bn_stats(out=stats[:], in_=src_ap)
                nc.vector.bn_aggr(out=mv[:], in_=stats[:])
                sd = tpool.tile([128, 1], FP, tag="sd", name="sd")
                nc.scalar.activation(out=sd[:], in_=mv[:, 1:2],
                                     func=mybir.ActivationFunctionType.Sqrt,
                                     bias=eps_t[:], scale=1.0)
                rs = tpool.tile([128, 1], FP, tag="rs", name="rs")
                nc.vector.reciprocal(out=rs[:], in_=sd[:])
                nc.vector.tensor_scalar(out=dst_ap, in0=src_ap,
                                        scalar1=mv[:, 0:1], scalar2=rs[:],
                                        op0=mybir.AluOpType.subtract,
                                        op1=mybir.AluOpType.mult)
                if gamma_row is not None:
                    nc.vector.tensor_mul(out=dst_ap, in0=dst_ap, in1=gamma_row[:])
                if beta_row is not None:
                    nc.vector.tensor_add(out=dst_ap, in0=dst_ap, in1=beta_row[:])

            # ---------------- front: h0 = LN(gelu(x@W1+b1)) -------------
            for t in range(tiles):
                xt = xpool.tile([128, 2, 128], FP, tag="xt")
                nc.sync.dma_start(
                    out=xt[:],
                    in_=xT[:].rearrange("(k p) n -> p k n", p=128)[:, :, t * 128:(t + 1) * 128])
                ps = ps_f.tile([128, 512], FP, tag="psf")
                for k in range(2):
                    nc.tensor.matmul(out=ps[:, :d], lhsT=xt[:, k, :], rhs=W1t[:, k, :],
                                     start=(k == 0), stop=(k == 1))
                ht = tpool.tile([128, d], FP, tag="ht")
                nc.vector.tensor_add(out=ht[:], in0=ps[:, :d], in1=b1r[:])
                nc.scalar.activation(out=ht[:], in_=ht[:],
                                     func=mybir.ActivationFunctionType.Gelu)
                layernorm(h_sb[:, t, :], ht[:], g1r, be1r)
                nc.scalar.mul(out=ah0_sb[:, t, :], in_=h_sb[:, t, :], mul=ALPHA)
                nc.vector.tensor_scalar_mul(out=g_sb[:, t, :], in0=h_sb[:, t, :],
                                            scalar1=dinv_t[:, t:t + 1])

            def relay_hop(hop):
                ai = ag_in[hop % 2]
                for pz in range(plan.n_passes):
                    t0 = pz * plan.pass_tiles
                    t1 = min(t0 + plan.pass_tiles, tiles)
                    lo = t0 * 128 * d
                    hi = lo + (t1 - t0) * 128 * d
                    nc.sync.dma_start(
                        out=ai[lo:hi].rearrange("(p x) -> p x", p=128),
                        in_=g_sb[:, t0:t1, :])
                nc.gpsimd.collective_compute(
                    "AllGather", mybir.AluOpType.bypass,
                    ins=[ai[:]], outs=[g_full[hop % 2][:]],
                    replica_groups=rg)

            relay_hop(0)

            # ---------------- hops ------------------------------------
            for hop in range(1, n_hops + 1):
                gf = g_full[(hop - 1) % 2]
                gf_v = gf[:].rearrange("(r x) -> r x", x=256)
                for pz in range(plan.n_passes):
                    t0 = pz * plan.pass_tiles
                    t1 = min(t0 + plan.pass_tiles, tiles)
                    ntile = t1 - t0
                    nbank = math.ceil(ntile / TILES_PER_BANK)
                    for b in range(nbank):
                        bt0 = t0 + b * TILES_PER_BANK
                        bt1 = min(bt0 + TILES_PER_BANK, t1)
                        bank = ps_s.tile([128, 512], FP, tag="seg",
                                         name=f"seg_{hop}_{pz}_{b}")
                        for tb in range(bt1 - bt0):
                            nc.tensor.matmul(out=bank[:, tb * 64:tb * 64 + 64],
                                             lhsT=ident_bf[:],
                                             rhs=g_sb[:, bt0 + tb, :],
                                             start=(tb == 0), stop=False,
                                             skip_group_check=True)
                        for w in range(2):
                            gk = (pz, b, w)
                            coff0 = pre["chunk_off"][gk]
                            for (bgk, lo, bc, coff) in pre["blocks"]:
                                if bgk != gk:
                                    continue
                                chunks = pre["grid"][gk][lo:lo + bc]
                                msg = mpool.tile([128, BLOCK_CHUNKS, 128], BF,
                                                 tag="msg")
                                nc.gpsimd.dma_gather(
                                    msg[:, :bc, :], gf_v[:, w * 128:(w + 1) * 128],
                                    idx_t[:, coff:coff + bc * 8],
                                    bc * 128, bc * 128, 128, elem_step=256,
                                    queue_num=gq[0] % GATHER_QUEUES)
                                gq[0] += 1
                                for ci, t in enumerate(chunks):
                                    tb = t - bt0
                                    cgi = coff0 + lo + ci
                                    sg = sgpool.tile([128, 256], BF, tag="sgen")
                                    nc.vector.tensor_scalar(
                                        out=sg[:], in0=iota_bf[:],
                                        scalar1=dr_t[:, cgi:cgi + 1],
                                        scalar2=None,
                                        op0=mybir.AluOpType.is_equal)
                                    for par in range(2):
                                        nc.tensor.matmul(
                                            out=bank[:, tb * 64:tb * 64 + 64],
                                            lhsT=sg[:, par * 128:par * 128 + 128],
                                            rhs=msg[:, ci, par * 64:par * 64 + 64],
                                            start=False, stop=False,
                                            skip_group_check=True)
                        for tb in range(bt1 - bt0):
                            t = bt0 + tb
                            nc.vector.tensor_scalar_mul(
                                out=h_sb[:, t, :],
                                in0=bank[:, tb * 64:tb * 64 + 64],
                                scalar1=dsc_t[:, t:t + 1])
                            nc.vector.tensor_add(out=h_sb[:, t, :],
                                                 in0=h_sb[:, t, :],
                                                 in1=ah0_sb[:, t, :])
                            nc.vector.tensor_scalar_mul(
                                out=g_sb[:, t, :], in0=h_sb[:, t, :],
                                scalar1=dinv_t[:, t:t + 1])
                if hop < n_hops:
                    relay_hop(hop)

            # ---------------- back: y = LN(gelu(h)) @ W2p + b2p ----------
            for t in range(tiles):
                gt = tpool.tile([128, d], FP, tag="gt")
                nc.scalar.activation(out=gt[:], in_=h_sb[:, t, :],
                                     func=mybir.ActivationFunctionType.Gelu)
                lt = tpool.tile([128, d], FP, tag="lt")
                layernorm(lt[:], gt[:], None, None)
                pst = ps_f.tile([128, 512], FP, tag="psf")
                nc.tensor.transpose(out=pst[:64, :128], in_=lt[:], identity=ident[:])
                htr = tpool.tile([64, 128], FP, tag="htr")
                nc.vector.tensor_copy(out=htr[:], in_=pst[:64, :128])
                pso = ps_f.tile([128, 512], FP, tag="psf")
                nc.tensor.matmul(out=pso[:, :OUT_C], lhsT=htr[:], rhs=W2t[:],
                                 start=True, stop=True)
                yt = tpool.tile([128, OUT_C], FP, tag="yt")
                nc.vector.tensor_add(out=yt[:], in0=pso[:, :OUT_C], in1=b2r[:])
                nc.sync.dma_start(out=y[t * 128:(t + 1) * 128, :], in_=yt[:])
    nc.finalize()
    return nc


def make_in_maps(pre, W1, b1, g1, be1, g2, be2, W2, b2):
    W2p = (np.asarray(g2)[:, None] * np.asarray(W2)).astype(np.float32)
    b2p = (np.asarray(be2) @ np.asarray(W2) + np.asarray(b2)).astype(np.float32)
    iota = np.arange(256, dtype=np.float32)
    in_maps = []
    for c in range(N_CORES):
        in_maps.append({
            "xT": pre["x_t"][c],
            "W1": np.asarray(W1, np.float32), "b1": np.asarray(b1, np.float32),
            "g1": np.asarray(g1, np.float32), "be1": np.asarray(be1, np.float32),
            "W2p": W2p, "b2p": b2p,
            "dinv": pre["dinv_t"][c], "dsc": pre["dsc_t"][c],
            "idxs": pre["idx_bufs"][c], "drow": pre["dr_bufs"][c],
            "iota": iota,
        })
    return in_maps


def kernel(x, edge_index, W1, b1, g1, be1, g2, be2, W2, b2):
    from concourse.bass_utils import run_bass_kernel_spmd
    x = np.asarray(x, np.float32)
    edge_index = np.asarray(edge_index)
    plan = Plan(N, E)
    pre = preprocess(x, edge_index, plan)
    nc = build_nc(plan, pre)
    in_maps = make_in_maps(pre, W1, b1, g1, be1, g2, be2, W2, b2)
    res = run_bass_kernel_spmd(nc, in_maps, core_ids=list(range(N_CORES)),
                               trace=False)
    pc = plan.per_core
    out = np.empty((N, OUT_C), np.float32)
    for c in range(N_CORES):
        out[c * pc:(c + 1) * pc] = res.results[c]["y"][:pc]
    return out
